# revision 12
# baseline (speedup 1.0000x reference)
"""Trainium2 Bass kernel for nn_LossMatch: loss = 80 * mean(|e[b,k,d] - W[d, i[b]]|).

Shapes: e_vectors [256, 32, 2048, 1] f32, W [2048, 100000] f32, i [256] int64.
Strategy: data-parallel over B across 8 cores (32 batch rows each). Only the
256 gathered columns of W are ever needed, so the host gathers W[:, i] and
ships each core its 32 target rows replicated 4x (to match the partition
layout below). Everything is shipped as bf16 to halve HBM traffic; the
per-element quantization is unbiased and averages out over 16.7M elements.

Per-core device layout: the 1024 (b, k) rows are tiled as 8 tiles of 128
partitions, tile t covering k in {4t..4t+3}, partition p = 4*b_local + (k-4t).
With that mapping every tile uses the same replicated target tile
trep[p] = target[p//4], so the target is loaded once.

Per tile: DVE tensor_tensor(subtract) bf16 (2x mode), then abs + row-sum in a
single op: tensor_scalar(abs_max, 0) with accum_out on DVE (4x mode) for half
the tiles, and ScalarE activation(Abs) with accum_out for the other half.
Per-core output is the [128, 8] matrix of partial sums; the host reduces in
float64 and applies the 80/count scaling.
"""

import numpy as np
import ml_dtypes

B, K, D = 256, 32, 2048
NCORES = 8
BPC = B // NCORES            # batch rows per core: 32
ROWS = BPC * K               # (b, k) rows per core: 1024
NTILES = ROWS // 128         # 8
MATCH_WEIGHT = 80.0

# Tiles reduced entirely on the DVE via the max/min identity
#   sum|e-t| = sum(max(e,t)) - sum(min(e,t))
# (two scalar_tensor_tensor ops with accum_out, no subtract pass); the rest
# use a DVE subtract followed by ScalarE activation(Abs) with accum_out.
# Split chosen to balance DVE vs ACT engine time.
MAXMIN_TILES = (0, 3, 6)

_cached = None


def _split_multiwaits(nc, max_waits=1):
    """The walrus build here rejects instructions carrying more than one sync
    wait. Split any multi-wait instruction into a chain of same-engine NOPs,
    each carrying one wait, placed immediately before it — semantically
    identical (the queue stalls on each wait in turn)."""
    import bass_rust

    for f in nc.m.functions:
        for bb in f.blocks:
            insts = bb.instructions
            fixups = []
            for idx, ins in enumerate(insts):
                si = ins.sync_info
                waits = list(si.on_wait) if si is not None and si.on_wait else []
                if len(waits) > max_waits:
                    fixups.append((idx, ins, waits))
            for idx, ins, waits in reversed(fixups):
                carried, kept = waits[:-max_waits], waits[-max_waits:]
                ins.sync_info.on_wait = kept
                nops = []
                for w in carried:
                    n = nc.engines[ins.engine].nop(nofuse=True)
                    n.ins.sync_info = bass_rust.SyncInfo(on_wait=[w], on_update=[])
                    # engine.nop() appended it to the current tail block;
                    # pull it back out and splice it in front of `ins`.
                    for b2 in f.blocks:
                        if n.ins in b2.instructions:
                            b2.instructions.remove(n.ins)
                    nops.append(n.ins)
                insts[idx:idx] = nops
    return nc


def _build_nc():
    import concourse.bass as bass
    import concourse.tile as tile
    from concourse import mybir

    nc = bass.Bass()
    e = nc.dram_tensor("e", [ROWS, D], mybir.dt.bfloat16, kind="ExternalInput")
    trep = nc.dram_tensor("trep", [128, D], mybir.dt.bfloat16, kind="ExternalInput")
    out = nc.dram_tensor("partials", [128, 2 * NTILES], mybir.dt.float32, kind="ExternalOutput")

    with tile.TileContext(nc) as tc:
        with (
            tc.tile_pool(name="singles", bufs=1) as singles,
            tc.tile_pool(name="epool", bufs=NTILES) as epool,
            tc.tile_pool(name="dpool", bufs=4) as dpool,
            tc.tile_pool(name="apool", bufs=2) as apool,
        ):
            trep_t = singles.tile([128, D], mybir.dt.bfloat16)
            nc.sync.dma_start(out=trep_t[:], in_=trep[:])
            # Columns t < NTILES hold sum(max) (or the ACT abs-sum); columns
            # NTILES+t hold sum(min) for max/min tiles (zero otherwise). The
            # host computes sum(cols[:NTILES]) - sum(cols[NTILES:]).
            partials = singles.tile([128, 2 * NTILES], mybir.dt.float32)
            nc.vector.memset(partials[:], 0.0)

            for t in range(NTILES):
                et = epool.tile([128, D], mybir.dt.bfloat16, tag="et")
                nc.sync.dma_start(out=et[:], in_=e[t * 128:(t + 1) * 128, :])

                if t in MAXMIN_TILES:
                    mx = dpool.tile([128, D], mybir.dt.bfloat16, tag="mx")
                    nc.vector.scalar_tensor_tensor(
                        out=mx[:], in0=et[:], scalar=0.0, in1=trep_t[:],
                        op0=mybir.AluOpType.bypass, op1=mybir.AluOpType.max,
                        accum_out=partials[:, t:t + 1],
                    )
                    mn = dpool.tile([128, D], mybir.dt.bfloat16, tag="mn")
                    nc.vector.scalar_tensor_tensor(
                        out=mn[:], in0=et[:], scalar=0.0, in1=trep_t[:],
                        op0=mybir.AluOpType.bypass, op1=mybir.AluOpType.min,
                        accum_out=partials[:, NTILES + t:NTILES + t + 1],
                    )
                else:
                    diff = dpool.tile([128, D], mybir.dt.bfloat16, tag="diff")
                    nc.vector.tensor_tensor(
                        out=diff[:], in0=et[:], in1=trep_t[:],
                        op=mybir.AluOpType.subtract,
                    )
                    absd = apool.tile([128, D], mybir.dt.bfloat16, tag="absd")
                    nc.scalar.activation(
                        out=absd[:], in_=diff[:],
                        func=mybir.ActivationFunctionType.Abs,
                        accum_out=partials[:, t:t + 1],
                    )

            nc.sync.dma_start(out=out[:], in_=partials[:])
    return _split_multiwaits(nc)


def _prepare_in_maps(e_vectors, W, i):
    e = np.asarray(e_vectors, dtype=np.float32).reshape(B, K, D)
    idx = np.asarray(i).astype(np.int64)
    target = np.ascontiguousarray(W[:, idx].T)  # [B, D] f32, target[b] = W[:, i[b]]

    # [core, t, b_local, j, d] so device rows are tile-major with p = 4*b + j.
    e_bf = (
        e.reshape(NCORES, BPC, K // 4, 4, D)
        .transpose(0, 2, 1, 3, 4)
        .reshape(NCORES, ROWS, D)
        .astype(ml_dtypes.bfloat16)
    )
    t_bf = target.astype(ml_dtypes.bfloat16)

    in_maps = []
    for c in range(NCORES):
        t_rep = np.repeat(t_bf[c * BPC:(c + 1) * BPC], 4, axis=0)  # [128, D]
        in_maps.append({
            "e": np.ascontiguousarray(e_bf[c]),
            "trep": np.ascontiguousarray(t_rep),
        })
    return in_maps


def _run(e_vectors, W, i, **spmd_kwargs):
    """Returns (loss: np.float32, BassKernelResults)."""
    global _cached
    from concourse.bass_utils import run_bass_kernel_spmd

    if _cached is None:
        _cached = _build_nc()
    in_maps = _prepare_in_maps(e_vectors, W, i)
    res = run_bass_kernel_spmd(_cached, in_maps, core_ids=list(range(NCORES)), **spmd_kwargs)
    total = 0.0
    for r in res.results:
        p = np.asarray(r["partials"], dtype=np.float64)
        total += p[:, :NTILES].sum() - p[:, NTILES:].sum()
    loss = MATCH_WEIGHT * total / float(B * K * D)
    return np.float32(loss), res


def kernel(e_vectors, W, i):
    loss, _ = _run(e_vectors, W, i)
    return loss


# revision 30
# speedup vs baseline: 3.1281x; 3.1281x over previous
"""Trainium2 Bass kernel for nn_LossMatch: loss = 80 * mean(|e[b,k,d] - W[d, i[b]]|).

Shapes: e_vectors [256, 32, 2048, 1] f32, W [2048, 100000] f32, i [256] int64.
Strategy: data-parallel over B across 8 cores (32 batch rows each). Only the
256 gathered columns of W are ever needed, so the host gathers W[:, i] and
ships each core its 32 target rows replicated 4x (to match the partition
layout below). Everything is shipped as bf16 to halve HBM traffic; the
per-element quantization is unbiased and averages out over 16.7M elements.

Per-core device layout: the 1024 (b, k) rows are tiled as 8 tiles of 128
partitions, tile t covering k in {4t..4t+3}, partition p = 4*b_local + (k-4t).
With that mapping every tile uses the same replicated target tile
trep[p] = target[p//4], so the target is loaded once.

Per tile: DVE tensor_tensor(subtract) bf16 (2x mode), then abs + row-sum in a
single op: tensor_scalar(abs_max, 0) with accum_out on DVE (4x mode) for half
the tiles, and ScalarE activation(Abs) with accum_out for the other half.
Per-core output is the [128, 8] matrix of partial sums; the host reduces in
float64 and applies the 80/count scaling.
"""

import numpy as np
import ml_dtypes

B, K, D = 256, 32, 2048
NCORES = 8
BPC = B // NCORES            # batch rows per core: 32
ROWS = BPC * K               # (b, k) rows per core: 1024
NTILES = ROWS // 128         # 8
MATCH_WEIGHT = 80.0

# Per-tile engine assignment, chosen to balance DVE / ACT / Pool / PE / DMA:
#   M: DVE max(e,t) + DVE min(e,t), PE ones-matmul partition-sums into two
#      PSUM accumulators (sum|e-t| = sum(max) - sum(min))
#   R: DVE tensor_tensor(sub) + DVE tensor_reduce(add, abs) -> partials col
#   A: DVE tensor_tensor(sub) + ScalarE activation(Abs, accum_out)
#   P: GPSIMD tensor_tensor(sub) + ScalarE activation(Abs, accum_out)
TILE_MODES = "RAPAPARR"
# The last tile is processed in this many column-chunks to shorten the
# critical path after its DMA lands (only applied when its mode is A).
TAIL_SPLITS = 1

_cached = None


def _split_multiwaits(nc, max_waits=1):
    """The walrus build here rejects instructions carrying more than one sync
    wait. Split any multi-wait instruction into a chain of same-engine NOPs,
    each carrying one wait, placed immediately before it — semantically
    identical (the queue stalls on each wait in turn)."""
    import bass_rust

    for f in nc.m.functions:
        for bb in f.blocks:
            insts = bb.instructions
            fixups = []
            for idx, ins in enumerate(insts):
                si = ins.sync_info
                waits = list(si.on_wait) if si is not None and si.on_wait else []
                if len(waits) > max_waits:
                    fixups.append((idx, ins, waits))
            for idx, ins, waits in reversed(fixups):
                carried, kept = waits[:-max_waits], waits[-max_waits:]
                ins.sync_info.on_wait = kept
                nops = []
                for w in carried:
                    n = nc.engines[ins.engine].nop(nofuse=True)
                    n.ins.sync_info = bass_rust.SyncInfo(on_wait=[w], on_update=[])
                    # engine.nop() appended it to the current tail block;
                    # pull it back out and splice it in front of `ins`.
                    for b2 in f.blocks:
                        if n.ins in b2.instructions:
                            b2.instructions.remove(n.ins)
                    nops.append(n.ins)
                insts[idx:idx] = nops
    return nc


def _build_nc(modes=None, tail_splits=None):
    import concourse.bass as bass
    import concourse.tile as tile
    from concourse import mybir

    modes = TILE_MODES if modes is None else modes
    tail_splits = TAIL_SPLITS if tail_splits is None else tail_splits
    m_tiles = [t for t in range(NTILES) if modes[t] == "M"]
    NMM = 512  # matmul free-dim chunk (one PSUM bank)

    p_cols = NTILES + max(tail_splits - 1, 1)
    nc = bass.Bass()
    e = nc.dram_tensor("e", [ROWS, D], mybir.dt.bfloat16, kind="ExternalInput")
    trep = nc.dram_tensor("trep", [128, D], mybir.dt.bfloat16, kind="ExternalInput")
    out = nc.dram_tensor("partials", [128, p_cols], mybir.dt.float32, kind="ExternalOutput")
    if m_tiles:
        pe_out = nc.dram_tensor("pe_out", [1, D], mybir.dt.float32, kind="ExternalOutput")

    with tile.TileContext(nc) as tc:
        with (
            tc.tile_pool(name="singles", bufs=1) as singles,
            tc.tile_pool(name="epool", bufs=NTILES) as epool,
            tc.tile_pool(name="dpool", bufs=4) as dpool,
            tc.tile_pool(name="mpool", bufs=4) as mpool,
            tc.tile_pool(name="apool", bufs=2) as apool,
            tc.tile_pool(name="pspool", bufs=1, space="PSUM") as pspool,
        ):
            trep_t = singles.tile([128, D], mybir.dt.bfloat16)
            nc.sync.dma_start(out=trep_t[:], in_=trep[:])
            partials = singles.tile([128, p_cols], mybir.dt.float32)
            if m_tiles:
                ones = singles.tile([128, 1], mybir.dt.bfloat16)
                nc.gpsimd.memset(ones[:], 1.0)
                neg_ones = singles.tile([128, 1], mybir.dt.bfloat16)
                nc.gpsimd.memset(neg_ones[:], -1.0)
                ps_acc = pspool.tile([1, D], mybir.dt.float32)

            for t in range(NTILES):
                mode = modes[t]
                et = epool.tile([128, D], mybir.dt.bfloat16, tag="et")
                nc.sync.dma_start(out=et[:], in_=e[t * 128:(t + 1) * 128, :])

                if mode == "M":
                    first = t == m_tiles[0]
                    last = t == m_tiles[-1]
                    mx = mpool.tile([128, D], mybir.dt.bfloat16, tag="mx")
                    nc.vector.tensor_tensor(
                        out=mx[:], in0=et[:], in1=trep_t[:], op=mybir.AluOpType.max)
                    mn = mpool.tile([128, D], mybir.dt.bfloat16, tag="mn")
                    nc.vector.tensor_tensor(
                        out=mn[:], in0=et[:], in1=trep_t[:], op=mybir.AluOpType.min)
                    # sum|e-t| = sum(1*max) + sum(-1*min), both into one PSUM
                    # accumulator via +/- ones as the stationary operand.
                    for j in range(D // NMM):
                        sl = slice(j * NMM, (j + 1) * NMM)
                        nc.tensor.matmul(ps_acc[:, sl], ones[:], mx[:, sl],
                                         start=first, stop=False)
                        nc.tensor.matmul(ps_acc[:, sl], neg_ones[:], mn[:, sl],
                                         start=False, stop=last)
                    continue

                if mode == "H":
                    # Hybrid: subtract on DVE, then reduce half on DVE and
                    # half on ACT in parallel — shortest tail for the last
                    # tile, whose DMA lands latest.
                    diff = dpool.tile([128, D], mybir.dt.bfloat16, tag="diffH")
                    nc.vector.tensor_tensor(
                        out=diff[:], in0=et[:], in1=trep_t[:],
                        op=mybir.AluOpType.subtract,
                    )
                    half = D // 2
                    nc.vector.tensor_reduce(
                        out=partials[:, t:t + 1], in_=diff[:, :half],
                        axis=mybir.AxisListType.X, op=mybir.AluOpType.add,
                        apply_absolute_value=True,
                    )
                    absd = apool.tile([128, half], mybir.dt.bfloat16, tag="absdH")
                    nc.scalar.activation(
                        out=absd[:], in_=diff[:, half:],
                        func=mybir.ActivationFunctionType.Abs,
                        accum_out=partials[:, NTILES:NTILES + 1],
                    )
                    continue

                chunks = tail_splits if (t == NTILES - 1 and mode == "A") else 1
                w = D // chunks
                for c in range(chunks):
                    sl = slice(c * w, (c + 1) * w)
                    diff = dpool.tile([128, w], mybir.dt.bfloat16,
                                      tag=f"diff{c}")
                    sub_engine = nc.gpsimd if mode == "P" else nc.vector
                    sub_engine.tensor_tensor(
                        out=diff[:], in0=et[:, sl], in1=trep_t[:, sl],
                        op=mybir.AluOpType.subtract,
                    )
                    col = t if c == 0 else NTILES + c - 1
                    acol = partials[:, col:col + 1]
                    if mode == "R":
                        nc.vector.tensor_reduce(
                            out=acol, in_=diff[:],
                            axis=mybir.AxisListType.X, op=mybir.AluOpType.add,
                            apply_absolute_value=True,
                        )
                    else:
                        absd = apool.tile([128, w], mybir.dt.bfloat16,
                                          tag=f"absd{c}")
                        nc.scalar.activation(
                            out=absd[:], in_=diff[:],
                            func=mybir.ActivationFunctionType.Abs,
                            accum_out=acol,
                        )

            if m_tiles:
                evac = singles.tile([1, D], mybir.dt.float32)
                nc.scalar.copy(out=evac[:], in_=ps_acc[:])
                nc.sync.dma_start(out=pe_out[:], in_=evac[:])
            nc.sync.dma_start(out=out[:], in_=partials[:])
    return _split_multiwaits(nc)


def _prepare_in_maps(e_vectors, W, i):
    e = np.asarray(e_vectors, dtype=np.float32).reshape(B, K, D)
    idx = np.asarray(i).astype(np.int64)
    target = np.ascontiguousarray(W[:, idx].T)  # [B, D] f32, target[b] = W[:, i[b]]

    # [core, t, b_local, j, d] so device rows are tile-major with p = 4*b + j.
    e_bf = (
        e.reshape(NCORES, BPC, K // 4, 4, D)
        .transpose(0, 2, 1, 3, 4)
        .reshape(NCORES, ROWS, D)
        .astype(ml_dtypes.bfloat16)
    )
    t_bf = target.astype(ml_dtypes.bfloat16)

    in_maps = []
    for c in range(NCORES):
        t_rep = np.repeat(t_bf[c * BPC:(c + 1) * BPC], 4, axis=0)  # [128, D]
        in_maps.append({
            "e": np.ascontiguousarray(e_bf[c]),
            "trep": np.ascontiguousarray(t_rep),
        })
    return in_maps


def _run(e_vectors, W, i, **spmd_kwargs):
    """Returns (loss: np.float32, BassKernelResults)."""
    global _cached
    from concourse.bass_utils import run_bass_kernel_spmd

    if _cached is None:
        _cached = _build_nc()
    in_maps = _prepare_in_maps(e_vectors, W, i)
    res = run_bass_kernel_spmd(_cached, in_maps, core_ids=list(range(NCORES)), **spmd_kwargs)
    total = 0.0
    for r in res.results:
        total += np.asarray(r["partials"], dtype=np.float64).sum()
        if "pe_out" in r:
            total += np.asarray(r["pe_out"], dtype=np.float64).sum()
    loss = MATCH_WEIGHT * total / float(B * K * D)
    return np.float32(loss), res


def kernel(e_vectors, W, i):
    loss, _ = _run(e_vectors, W, i)
    return loss


# revision 32
# speedup vs baseline: 11.3295x; 3.6219x over previous
"""Trainium2 Bass kernel for nn_LossMatch: loss = 80 * mean(|e[b,k,d] - W[d, i[b]]|).

Shapes: e_vectors [256, 32, 2048, 1] f32, W [2048, 100000] f32, i [256] int64.
Strategy: data-parallel over B across 8 cores (32 batch rows each). Only the
256 gathered columns of W are ever needed, so the host gathers W[:, i] and
ships each core its 32 target rows replicated 4x (to match the partition
layout below). Everything is shipped as bf16 to halve HBM traffic; the
per-element quantization is unbiased and averages out over 16.7M elements.

Per-core device layout: the 1024 (b, k) rows are tiled as 8 tiles of 128
partitions, tile t covering k in {4t..4t+3}, partition p = 4*b_local + (k-4t).
With that mapping every tile uses the same replicated target tile
trep[p] = target[p//4], so the target is loaded once.

Per tile: DVE tensor_tensor(subtract) bf16 (2x mode), then abs + row-sum in a
single op: tensor_scalar(abs_max, 0) with accum_out on DVE (4x mode) for half
the tiles, and ScalarE activation(Abs) with accum_out for the other half.
Per-core output is the [128, 8] matrix of partial sums; the host reduces in
float64 and applies the 80/count scaling.
"""

import numpy as np
import ml_dtypes

B, K, D = 256, 32, 2048
NCORES = 8
BPC = B // NCORES            # batch rows per core: 32
ROWS = BPC * K               # (b, k) rows per core: 1024
NTILES = ROWS // 128         # 8
MATCH_WEIGHT = 80.0

# Per-tile engine assignment, chosen to balance DVE / ACT / Pool / PE / DMA:
#   M: DVE max(e,t) + DVE min(e,t), PE ones-matmul partition-sums into two
#      PSUM accumulators (sum|e-t| = sum(max) - sum(min))
#   R: DVE tensor_tensor(sub) + DVE tensor_reduce(add, abs) -> partials col
#   A: DVE tensor_tensor(sub) + ScalarE activation(Abs, accum_out)
#   P: GPSIMD tensor_tensor(sub) + ScalarE activation(Abs, accum_out)
TILE_MODES = "RAPAPARR"
# The last tile is processed in this many column-chunks to shorten the
# critical path after its DMA lands (only applied when its mode is A).
TAIL_SPLITS = 1

_cached = None


def _split_multiwaits(nc, max_waits=1):
    """The walrus build here rejects instructions carrying more than one sync
    wait. Split any multi-wait instruction into a chain of same-engine NOPs,
    each carrying one wait, placed immediately before it — semantically
    identical (the queue stalls on each wait in turn)."""
    import bass_rust

    for f in nc.m.functions:
        for bb in f.blocks:
            insts = bb.instructions
            fixups = []
            for idx, ins in enumerate(insts):
                si = ins.sync_info
                waits = list(si.on_wait) if si is not None and si.on_wait else []
                if len(waits) > max_waits:
                    fixups.append((idx, ins, waits))
            for idx, ins, waits in reversed(fixups):
                carried, kept = waits[:-max_waits], waits[-max_waits:]
                ins.sync_info.on_wait = kept
                nops = []
                for w in carried:
                    n = nc.engines[ins.engine].nop(nofuse=True)
                    n.ins.sync_info = bass_rust.SyncInfo(on_wait=[w], on_update=[])
                    # engine.nop() appended it to the current tail block;
                    # pull it back out and splice it in front of `ins`.
                    for b2 in f.blocks:
                        if n.ins in b2.instructions:
                            b2.instructions.remove(n.ins)
                    nops.append(n.ins)
                insts[idx:idx] = nops
    return nc


def _build_nc(modes=None, tail_splits=None, unroll=1):
    """unroll > 1 repeats the whole per-core body (same inputs, same outputs)
    back-to-back; used only for steady-state HW timing, where the wall-clock
    delta between unroll=K and unroll=1 isolates K-1 kernel iterations from
    dispatch noise."""
    import concourse.bass as bass
    import concourse.tile as tile
    from concourse import mybir

    modes = TILE_MODES if modes is None else modes
    tail_splits = TAIL_SPLITS if tail_splits is None else tail_splits
    m_tiles = [t for t in range(NTILES) if modes[t] == "M"]
    NMM = 512  # matmul free-dim chunk (one PSUM bank)

    p_cols = NTILES + max(tail_splits - 1, 1)
    nc = bass.Bass()
    e = nc.dram_tensor("e", [ROWS, D], mybir.dt.bfloat16, kind="ExternalInput")
    trep = nc.dram_tensor("trep", [128, D], mybir.dt.bfloat16, kind="ExternalInput")
    out = nc.dram_tensor("partials", [128, p_cols], mybir.dt.float32, kind="ExternalOutput")
    if m_tiles:
        pe_out = nc.dram_tensor("pe_out", [1, D], mybir.dt.float32, kind="ExternalOutput")

    with tile.TileContext(nc) as tc:
        with (
            tc.tile_pool(name="singles", bufs=1) as singles,
            tc.tile_pool(name="epool", bufs=NTILES) as epool,
            tc.tile_pool(name="dpool", bufs=4) as dpool,
            tc.tile_pool(name="mpool", bufs=4) as mpool,
            tc.tile_pool(name="apool", bufs=2) as apool,
            tc.tile_pool(name="pspool", bufs=1, space="PSUM") as pspool,
        ):
            trep_t = singles.tile([128, D], mybir.dt.bfloat16)
            nc.sync.dma_start(out=trep_t[:], in_=trep[:])
            partials = singles.tile([128, p_cols], mybir.dt.float32)
            if m_tiles:
                ones = singles.tile([128, 1], mybir.dt.bfloat16)
                nc.gpsimd.memset(ones[:], 1.0)
                neg_ones = singles.tile([128, 1], mybir.dt.bfloat16)
                nc.gpsimd.memset(neg_ones[:], -1.0)
                ps_acc = pspool.tile([1, D], mybir.dt.float32)

            for rep in range(unroll):
              for t in range(NTILES):
                mode = modes[t]
                et = epool.tile([128, D], mybir.dt.bfloat16, tag="et")
                nc.sync.dma_start(out=et[:], in_=e[t * 128:(t + 1) * 128, :])

                if mode == "M":
                    first = t == m_tiles[0]
                    last = t == m_tiles[-1]
                    mx = mpool.tile([128, D], mybir.dt.bfloat16, tag="mx")
                    nc.vector.tensor_tensor(
                        out=mx[:], in0=et[:], in1=trep_t[:], op=mybir.AluOpType.max)
                    mn = mpool.tile([128, D], mybir.dt.bfloat16, tag="mn")
                    nc.vector.tensor_tensor(
                        out=mn[:], in0=et[:], in1=trep_t[:], op=mybir.AluOpType.min)
                    # sum|e-t| = sum(1*max) + sum(-1*min), both into one PSUM
                    # accumulator via +/- ones as the stationary operand.
                    for j in range(D // NMM):
                        sl = slice(j * NMM, (j + 1) * NMM)
                        nc.tensor.matmul(ps_acc[:, sl], ones[:], mx[:, sl],
                                         start=first, stop=False)
                        nc.tensor.matmul(ps_acc[:, sl], neg_ones[:], mn[:, sl],
                                         start=False, stop=last)
                    continue

                if mode == "H":
                    # Hybrid: subtract on DVE, then reduce half on DVE and
                    # half on ACT in parallel — shortest tail for the last
                    # tile, whose DMA lands latest.
                    diff = dpool.tile([128, D], mybir.dt.bfloat16, tag="diffH")
                    nc.vector.tensor_tensor(
                        out=diff[:], in0=et[:], in1=trep_t[:],
                        op=mybir.AluOpType.subtract,
                    )
                    half = D // 2
                    nc.vector.tensor_reduce(
                        out=partials[:, t:t + 1], in_=diff[:, :half],
                        axis=mybir.AxisListType.X, op=mybir.AluOpType.add,
                        apply_absolute_value=True,
                    )
                    absd = apool.tile([128, half], mybir.dt.bfloat16, tag="absdH")
                    nc.scalar.activation(
                        out=absd[:], in_=diff[:, half:],
                        func=mybir.ActivationFunctionType.Abs,
                        accum_out=partials[:, NTILES:NTILES + 1],
                    )
                    continue

                chunks = tail_splits if (t == NTILES - 1 and mode == "A") else 1
                w = D // chunks
                for c in range(chunks):
                    sl = slice(c * w, (c + 1) * w)
                    diff = dpool.tile([128, w], mybir.dt.bfloat16,
                                      tag=f"diff{c}")
                    sub_engine = nc.gpsimd if mode == "P" else nc.vector
                    sub_engine.tensor_tensor(
                        out=diff[:], in0=et[:, sl], in1=trep_t[:, sl],
                        op=mybir.AluOpType.subtract,
                    )
                    col = t if c == 0 else NTILES + c - 1
                    acol = partials[:, col:col + 1]
                    if mode == "R":
                        nc.vector.tensor_reduce(
                            out=acol, in_=diff[:],
                            axis=mybir.AxisListType.X, op=mybir.AluOpType.add,
                            apply_absolute_value=True,
                        )
                    else:
                        absd = apool.tile([128, w], mybir.dt.bfloat16,
                                          tag=f"absd{c}")
                        nc.scalar.activation(
                            out=absd[:], in_=diff[:],
                            func=mybir.ActivationFunctionType.Abs,
                            accum_out=acol,
                        )

            if m_tiles:
                evac = singles.tile([1, D], mybir.dt.float32)
                nc.scalar.copy(out=evac[:], in_=ps_acc[:])
                nc.sync.dma_start(out=pe_out[:], in_=evac[:])
            nc.sync.dma_start(out=out[:], in_=partials[:])
    return _split_multiwaits(nc)


def _prepare_in_maps(e_vectors, W, i):
    e = np.asarray(e_vectors, dtype=np.float32).reshape(B, K, D)
    idx = np.asarray(i).astype(np.int64)
    target = np.ascontiguousarray(W[:, idx].T)  # [B, D] f32, target[b] = W[:, i[b]]

    # [core, t, b_local, j, d] so device rows are tile-major with p = 4*b + j.
    e_bf = (
        e.reshape(NCORES, BPC, K // 4, 4, D)
        .transpose(0, 2, 1, 3, 4)
        .reshape(NCORES, ROWS, D)
        .astype(ml_dtypes.bfloat16)
    )
    t_bf = target.astype(ml_dtypes.bfloat16)

    in_maps = []
    for c in range(NCORES):
        t_rep = np.repeat(t_bf[c * BPC:(c + 1) * BPC], 4, axis=0)  # [128, D]
        in_maps.append({
            "e": np.ascontiguousarray(e_bf[c]),
            "trep": np.ascontiguousarray(t_rep),
        })
    return in_maps


def _run(e_vectors, W, i, **spmd_kwargs):
    """Returns (loss: np.float32, BassKernelResults)."""
    global _cached
    from concourse.bass_utils import run_bass_kernel_spmd

    if _cached is None:
        _cached = _build_nc()
    in_maps = _prepare_in_maps(e_vectors, W, i)
    res = run_bass_kernel_spmd(_cached, in_maps, core_ids=list(range(NCORES)), **spmd_kwargs)
    total = 0.0
    for r in res.results:
        total += np.asarray(r["partials"], dtype=np.float64).sum()
        if "pe_out" in r:
            total += np.asarray(r["pe_out"], dtype=np.float64).sum()
    loss = MATCH_WEIGHT * total / float(B * K * D)
    return np.float32(loss), res


def kernel(e_vectors, W, i):
    loss, _ = _run(e_vectors, W, i)
    return loss


# revision 33
# speedup vs baseline: 11.8075x; 1.0422x over previous
"""Trainium2 Bass kernel for nn_LossMatch: loss = 80 * mean(|e[b,k,d] - W[d, i[b]]|).

Shapes: e_vectors [256, 32, 2048, 1] f32, W [2048, 100000] f32, i [256] int64.
Strategy: data-parallel over B across 8 cores (32 batch rows each). Only the
256 gathered columns of W are ever needed, so the host gathers W[:, i] and
ships each core its 32 target rows replicated 4x (to match the partition
layout below). Everything is shipped as bf16 to halve HBM traffic; the
per-element quantization is unbiased and averages out over 16.7M elements.

Per-core device layout: the 1024 (b, k) rows are tiled as 8 tiles of 128
partitions, tile t covering k in {4t..4t+3}, partition p = 4*b_local + (k-4t).
With that mapping every tile uses the same replicated target tile
trep[p] = target[p//4], so the target is loaded once.

Per tile: DVE tensor_tensor(subtract) bf16 (2x mode), then abs + row-sum in a
single op: tensor_scalar(abs_max, 0) with accum_out on DVE (4x mode) for half
the tiles, and ScalarE activation(Abs) with accum_out for the other half.
Per-core output is the [128, 8] matrix of partial sums; the host reduces in
float64 and applies the 80/count scaling.
"""

import numpy as np
import ml_dtypes

B, K, D = 256, 32, 2048
NCORES = 8
BPC = B // NCORES            # batch rows per core: 32
ROWS = BPC * K               # (b, k) rows per core: 1024
NTILES = ROWS // 128         # 8
MATCH_WEIGHT = 80.0

# Per-tile engine assignment, chosen to balance DVE / ACT / Pool / PE / DMA:
#   M: DVE max(e,t) + DVE min(e,t), PE ones-matmul partition-sums into two
#      PSUM accumulators (sum|e-t| = sum(max) - sum(min))
#   R: DVE tensor_tensor(sub) + DVE tensor_reduce(add, abs) -> partials col
#   A: DVE tensor_tensor(sub) + ScalarE activation(Abs, accum_out)
#   P: GPSIMD tensor_tensor(sub) + ScalarE activation(Abs, accum_out)
TILE_MODES = "ARAAPRRA"  # best of a TimelineSim search over mode strings
# The last tile is processed in this many column-chunks to shorten the
# critical path after its DMA lands (only applied when its mode is A).
TAIL_SPLITS = 1

_cached = None


def _split_multiwaits(nc, max_waits=1):
    """The walrus build here rejects instructions carrying more than one sync
    wait. Split any multi-wait instruction into a chain of same-engine NOPs,
    each carrying one wait, placed immediately before it — semantically
    identical (the queue stalls on each wait in turn)."""
    import bass_rust

    for f in nc.m.functions:
        for bb in f.blocks:
            insts = bb.instructions
            fixups = []
            for idx, ins in enumerate(insts):
                si = ins.sync_info
                waits = list(si.on_wait) if si is not None and si.on_wait else []
                if len(waits) > max_waits:
                    fixups.append((idx, ins, waits))
            for idx, ins, waits in reversed(fixups):
                carried, kept = waits[:-max_waits], waits[-max_waits:]
                ins.sync_info.on_wait = kept
                nops = []
                for w in carried:
                    n = nc.engines[ins.engine].nop(nofuse=True)
                    n.ins.sync_info = bass_rust.SyncInfo(on_wait=[w], on_update=[])
                    # engine.nop() appended it to the current tail block;
                    # pull it back out and splice it in front of `ins`.
                    for b2 in f.blocks:
                        if n.ins in b2.instructions:
                            b2.instructions.remove(n.ins)
                    nops.append(n.ins)
                insts[idx:idx] = nops
    return nc


def _build_nc(modes=None, tail_splits=None, unroll=1):
    """unroll > 1 repeats the whole per-core body (same inputs, same outputs)
    back-to-back; used only for steady-state HW timing, where the wall-clock
    delta between unroll=K and unroll=1 isolates K-1 kernel iterations from
    dispatch noise."""
    import concourse.bass as bass
    import concourse.tile as tile
    from concourse import mybir

    modes = TILE_MODES if modes is None else modes
    tail_splits = TAIL_SPLITS if tail_splits is None else tail_splits
    m_tiles = [t for t in range(NTILES) if modes[t] == "M"]
    NMM = 512  # matmul free-dim chunk (one PSUM bank)

    p_cols = NTILES + max(tail_splits - 1, 1)
    nc = bass.Bass()
    e = nc.dram_tensor("e", [ROWS, D], mybir.dt.bfloat16, kind="ExternalInput")
    trep = nc.dram_tensor("trep", [128, D], mybir.dt.bfloat16, kind="ExternalInput")
    out = nc.dram_tensor("partials", [128, p_cols], mybir.dt.float32, kind="ExternalOutput")
    if m_tiles:
        pe_out = nc.dram_tensor("pe_out", [1, D], mybir.dt.float32, kind="ExternalOutput")

    with tile.TileContext(nc) as tc:
        with (
            tc.tile_pool(name="singles", bufs=1) as singles,
            tc.tile_pool(name="epool", bufs=NTILES) as epool,
            tc.tile_pool(name="dpool", bufs=4) as dpool,
            tc.tile_pool(name="mpool", bufs=4) as mpool,
            tc.tile_pool(name="apool", bufs=2) as apool,
            tc.tile_pool(name="pspool", bufs=1, space="PSUM") as pspool,
        ):
            trep_t = singles.tile([128, D], mybir.dt.bfloat16)
            nc.sync.dma_start(out=trep_t[:], in_=trep[:])
            partials = singles.tile([128, p_cols], mybir.dt.float32)
            if m_tiles:
                ones = singles.tile([128, 1], mybir.dt.bfloat16)
                nc.gpsimd.memset(ones[:], 1.0)
                neg_ones = singles.tile([128, 1], mybir.dt.bfloat16)
                nc.gpsimd.memset(neg_ones[:], -1.0)
                ps_acc = pspool.tile([1, D], mybir.dt.float32)

            for rep in range(unroll):
              for t in range(NTILES):
                mode = modes[t]
                et = epool.tile([128, D], mybir.dt.bfloat16, tag="et")
                nc.sync.dma_start(out=et[:], in_=e[t * 128:(t + 1) * 128, :])

                if mode == "M":
                    first = t == m_tiles[0]
                    last = t == m_tiles[-1]
                    mx = mpool.tile([128, D], mybir.dt.bfloat16, tag="mx")
                    nc.vector.tensor_tensor(
                        out=mx[:], in0=et[:], in1=trep_t[:], op=mybir.AluOpType.max)
                    mn = mpool.tile([128, D], mybir.dt.bfloat16, tag="mn")
                    nc.vector.tensor_tensor(
                        out=mn[:], in0=et[:], in1=trep_t[:], op=mybir.AluOpType.min)
                    # sum|e-t| = sum(1*max) + sum(-1*min), both into one PSUM
                    # accumulator via +/- ones as the stationary operand.
                    for j in range(D // NMM):
                        sl = slice(j * NMM, (j + 1) * NMM)
                        nc.tensor.matmul(ps_acc[:, sl], ones[:], mx[:, sl],
                                         start=first, stop=False)
                        nc.tensor.matmul(ps_acc[:, sl], neg_ones[:], mn[:, sl],
                                         start=False, stop=last)
                    continue

                if mode == "H":
                    # Hybrid: subtract on DVE, then reduce half on DVE and
                    # half on ACT in parallel — shortest tail for the last
                    # tile, whose DMA lands latest.
                    diff = dpool.tile([128, D], mybir.dt.bfloat16, tag="diffH")
                    nc.vector.tensor_tensor(
                        out=diff[:], in0=et[:], in1=trep_t[:],
                        op=mybir.AluOpType.subtract,
                    )
                    half = D // 2
                    nc.vector.tensor_reduce(
                        out=partials[:, t:t + 1], in_=diff[:, :half],
                        axis=mybir.AxisListType.X, op=mybir.AluOpType.add,
                        apply_absolute_value=True,
                    )
                    absd = apool.tile([128, half], mybir.dt.bfloat16, tag="absdH")
                    nc.scalar.activation(
                        out=absd[:], in_=diff[:, half:],
                        func=mybir.ActivationFunctionType.Abs,
                        accum_out=partials[:, NTILES:NTILES + 1],
                    )
                    continue

                chunks = tail_splits if (t == NTILES - 1 and mode == "A") else 1
                w = D // chunks
                for c in range(chunks):
                    sl = slice(c * w, (c + 1) * w)
                    diff = dpool.tile([128, w], mybir.dt.bfloat16,
                                      tag=f"diff{c}")
                    sub_engine = nc.gpsimd if mode == "P" else nc.vector
                    sub_engine.tensor_tensor(
                        out=diff[:], in0=et[:, sl], in1=trep_t[:, sl],
                        op=mybir.AluOpType.subtract,
                    )
                    col = t if c == 0 else NTILES + c - 1
                    acol = partials[:, col:col + 1]
                    if mode == "R":
                        nc.vector.tensor_reduce(
                            out=acol, in_=diff[:],
                            axis=mybir.AxisListType.X, op=mybir.AluOpType.add,
                            apply_absolute_value=True,
                        )
                    else:
                        absd = apool.tile([128, w], mybir.dt.bfloat16,
                                          tag=f"absd{c}")
                        nc.scalar.activation(
                            out=absd[:], in_=diff[:],
                            func=mybir.ActivationFunctionType.Abs,
                            accum_out=acol,
                        )

            if m_tiles:
                evac = singles.tile([1, D], mybir.dt.float32)
                nc.scalar.copy(out=evac[:], in_=ps_acc[:])
                nc.sync.dma_start(out=pe_out[:], in_=evac[:])
            nc.sync.dma_start(out=out[:], in_=partials[:])
    return _split_multiwaits(nc)


def _prepare_in_maps(e_vectors, W, i):
    e = np.asarray(e_vectors, dtype=np.float32).reshape(B, K, D)
    idx = np.asarray(i).astype(np.int64)
    target = np.ascontiguousarray(W[:, idx].T)  # [B, D] f32, target[b] = W[:, i[b]]

    # [core, t, b_local, j, d] so device rows are tile-major with p = 4*b + j.
    e_bf = (
        e.reshape(NCORES, BPC, K // 4, 4, D)
        .transpose(0, 2, 1, 3, 4)
        .reshape(NCORES, ROWS, D)
        .astype(ml_dtypes.bfloat16)
    )
    t_bf = target.astype(ml_dtypes.bfloat16)

    in_maps = []
    for c in range(NCORES):
        t_rep = np.repeat(t_bf[c * BPC:(c + 1) * BPC], 4, axis=0)  # [128, D]
        in_maps.append({
            "e": np.ascontiguousarray(e_bf[c]),
            "trep": np.ascontiguousarray(t_rep),
        })
    return in_maps


def _run(e_vectors, W, i, **spmd_kwargs):
    """Returns (loss: np.float32, BassKernelResults)."""
    global _cached
    from concourse.bass_utils import run_bass_kernel_spmd

    if _cached is None:
        _cached = _build_nc()
    in_maps = _prepare_in_maps(e_vectors, W, i)
    res = run_bass_kernel_spmd(_cached, in_maps, core_ids=list(range(NCORES)), **spmd_kwargs)
    total = 0.0
    for r in res.results:
        total += np.asarray(r["partials"], dtype=np.float64).sum()
        if "pe_out" in r:
            total += np.asarray(r["pe_out"], dtype=np.float64).sum()
    loss = MATCH_WEIGHT * total / float(B * K * D)
    return np.float32(loss), res


def kernel(e_vectors, W, i):
    loss, _ = _run(e_vectors, W, i)
    return loss


# revision 38
# speedup vs baseline: 31.9032x; 2.7019x over previous
"""Trainium2 Bass kernel for nn_LossMatch: loss = 80 * mean(|e[b,k,d] - W[d, i[b]]|).

Shapes: e_vectors [256, 32, 2048, 1] f32, W [2048, 100000] f32, i [256] int64.
Strategy: data-parallel over B across 8 cores (32 batch rows each). Only the
256 gathered columns of W are ever needed, so the host gathers W[:, i] and
ships each core its 32 target rows replicated 4x (to match the partition
layout below). Everything is shipped as bf16 to halve HBM traffic; the
per-element quantization is unbiased and averages out over 16.7M elements.

Per-core device layout: the 1024 (b, k) rows are tiled as 8 tiles of 128
partitions, tile t covering k in {4t..4t+3}, partition p = 4*b_local + (k-4t).
With that mapping every tile uses the same replicated target tile
trep[p] = target[p//4], so the target is loaded once.

Per tile: DVE tensor_tensor(subtract) bf16 (2x mode), then abs + row-sum in a
single op: tensor_scalar(abs_max, 0) with accum_out on DVE (4x mode) for half
the tiles, and ScalarE activation(Abs) with accum_out for the other half.
Per-core output is the [128, 8] matrix of partial sums; the host reduces in
float64 and applies the 80/count scaling.
"""

import numpy as np
import ml_dtypes

B, K, D = 256, 32, 2048
NCORES = 8
BPC = B // NCORES            # batch rows per core: 32
ROWS = BPC * K               # (b, k) rows per core: 1024
NTILES = ROWS // 128         # 8
MATCH_WEIGHT = 80.0

# Per-tile engine assignment, chosen to balance DVE / ACT / Pool / PE / DMA:
#   M: DVE max(e,t) + DVE min(e,t), PE ones-matmul partition-sums into two
#      PSUM accumulators (sum|e-t| = sum(max) - sum(min))
#   R: DVE tensor_tensor(sub) + DVE tensor_reduce(add, abs) -> partials col
#   A: DVE tensor_tensor(sub) + ScalarE activation(Abs, accum_out)
#   P: GPSIMD tensor_tensor(sub) + ScalarE activation(Abs, accum_out)
TILE_MODES = "AMAAPMRA"  # best of a TimelineSim search over mode strings
# The last tile is processed in this many column-chunks to shorten the
# critical path after its DMA lands (only applied when its mode is A).
TAIL_SPLITS = 1

_cached = None


def _split_multiwaits(nc, max_waits=1):
    """The walrus build here rejects instructions carrying more than one sync
    wait. Split any multi-wait instruction into a chain of same-engine NOPs,
    each carrying one wait, placed immediately before it — semantically
    identical (the queue stalls on each wait in turn)."""
    import bass_rust

    for f in nc.m.functions:
        for bb in f.blocks:
            insts = bb.instructions
            fixups = []
            for idx, ins in enumerate(insts):
                si = ins.sync_info
                waits = list(si.on_wait) if si is not None and si.on_wait else []
                if len(waits) > max_waits:
                    fixups.append((idx, ins, waits))
            for idx, ins, waits in reversed(fixups):
                carried, kept = waits[:-max_waits], waits[-max_waits:]
                ins.sync_info.on_wait = kept
                nops = []
                for w in carried:
                    n = nc.engines[ins.engine].nop(nofuse=True)
                    n.ins.sync_info = bass_rust.SyncInfo(on_wait=[w], on_update=[])
                    # engine.nop() appended it to the current tail block;
                    # pull it back out and splice it in front of `ins`.
                    for b2 in f.blocks:
                        if n.ins in b2.instructions:
                            b2.instructions.remove(n.ins)
                    nops.append(n.ins)
                insts[idx:idx] = nops
    return nc


def _build_nc(modes=None, tail_splits=None, unroll=1):
    """unroll > 1 repeats the whole per-core body (same inputs, same outputs)
    back-to-back; used only for steady-state HW timing, where the wall-clock
    delta between unroll=K and unroll=1 isolates K-1 kernel iterations from
    dispatch noise."""
    import concourse.bass as bass
    import concourse.tile as tile
    from concourse import mybir

    modes = TILE_MODES if modes is None else modes
    tail_splits = TAIL_SPLITS if tail_splits is None else tail_splits
    m_tiles = [t for t in range(NTILES) if modes[t] == "M"]
    NMM = 512  # matmul free-dim chunk (one PSUM bank)

    p_cols = NTILES + max(tail_splits - 1, 1)
    nc = bass.Bass()
    e = nc.dram_tensor("e", [ROWS, D], mybir.dt.bfloat16, kind="ExternalInput")
    trep = nc.dram_tensor("trep", [128, D], mybir.dt.bfloat16, kind="ExternalInput")
    out = nc.dram_tensor("partials", [128, p_cols], mybir.dt.float32, kind="ExternalOutput")
    if m_tiles:
        pe_out = nc.dram_tensor("pe_out", [1, NMM], mybir.dt.float32, kind="ExternalOutput")

    with tile.TileContext(nc) as tc:
        with (
            tc.tile_pool(name="singles", bufs=1) as singles,
            tc.tile_pool(name="epool", bufs=NTILES) as epool,
            tc.tile_pool(name="dpool", bufs=4) as dpool,
            tc.tile_pool(name="mpool", bufs=4) as mpool,
            tc.tile_pool(name="apool", bufs=2) as apool,
            tc.tile_pool(name="pspool", bufs=1, space="PSUM") as pspool,
        ):
            trep_t = singles.tile([128, D], mybir.dt.bfloat16)
            nc.sync.dma_start(out=trep_t[:], in_=trep[:])
            partials = singles.tile([128, p_cols], mybir.dt.float32)
            if m_tiles:
                ones = singles.tile([128, 1], mybir.dt.bfloat16)
                nc.gpsimd.memset(ones[:], 1.0)
                neg_ones = singles.tile([128, 1], mybir.dt.bfloat16)
                nc.gpsimd.memset(neg_ones[:], -1.0)
                # One PSUM bank accumulates everything: partitions via the
                # matmul contraction, column-slices and tiles via +=.
                ps_acc = pspool.tile([1, NMM], mybir.dt.float32)

            for rep in range(unroll):
              for t in range(NTILES):
                mode = modes[t]
                et = epool.tile([128, D], mybir.dt.bfloat16, tag="et")
                nc.sync.dma_start(out=et[:], in_=e[t * 128:(t + 1) * 128, :])

                if mode == "M":
                    first = t == m_tiles[0]
                    last = t == m_tiles[-1]
                    mx = mpool.tile([128, D], mybir.dt.bfloat16, tag="mx")
                    nc.vector.tensor_tensor(
                        out=mx[:], in0=et[:], in1=trep_t[:], op=mybir.AluOpType.max)
                    mn = mpool.tile([128, D], mybir.dt.bfloat16, tag="mn")
                    nc.vector.tensor_tensor(
                        out=mn[:], in0=et[:], in1=trep_t[:], op=mybir.AluOpType.min)
                    # sum|e-t| = sum(1*max) + sum(-1*min), all slices and all
                    # M tiles += into the same [1, NMM] PSUM bank.
                    nslices = D // NMM
                    for j in range(nslices):
                        sl = slice(j * NMM, (j + 1) * NMM)
                        nc.tensor.matmul(ps_acc[:], ones[:], mx[:, sl],
                                         start=(first and j == 0), stop=False)
                        nc.tensor.matmul(ps_acc[:], neg_ones[:], mn[:, sl],
                                         start=False,
                                         stop=(last and j == nslices - 1))
                    continue

                if mode == "H":
                    # Hybrid: subtract on DVE, then reduce half on DVE and
                    # half on ACT in parallel — shortest tail for the last
                    # tile, whose DMA lands latest.
                    diff = dpool.tile([128, D], mybir.dt.bfloat16, tag="diffH")
                    nc.vector.tensor_tensor(
                        out=diff[:], in0=et[:], in1=trep_t[:],
                        op=mybir.AluOpType.subtract,
                    )
                    half = D // 2
                    nc.vector.tensor_reduce(
                        out=partials[:, t:t + 1], in_=diff[:, :half],
                        axis=mybir.AxisListType.X, op=mybir.AluOpType.add,
                        apply_absolute_value=True,
                    )
                    absd = apool.tile([128, half], mybir.dt.bfloat16, tag="absdH")
                    nc.scalar.activation(
                        out=absd[:], in_=diff[:, half:],
                        func=mybir.ActivationFunctionType.Abs,
                        accum_out=partials[:, NTILES:NTILES + 1],
                    )
                    continue

                chunks = tail_splits if (t == NTILES - 1 and mode == "A") else 1
                w = D // chunks
                for c in range(chunks):
                    sl = slice(c * w, (c + 1) * w)
                    diff = dpool.tile([128, w], mybir.dt.bfloat16,
                                      tag=f"diff{c}")
                    sub_engine = nc.gpsimd if mode == "P" else nc.vector
                    sub_engine.tensor_tensor(
                        out=diff[:], in0=et[:, sl], in1=trep_t[:, sl],
                        op=mybir.AluOpType.subtract,
                    )
                    col = t if c == 0 else NTILES + c - 1
                    acol = partials[:, col:col + 1]
                    if mode == "R":
                        nc.vector.tensor_reduce(
                            out=acol, in_=diff[:],
                            axis=mybir.AxisListType.X, op=mybir.AluOpType.add,
                            apply_absolute_value=True,
                        )
                    else:
                        absd = apool.tile([128, w], mybir.dt.bfloat16,
                                          tag=f"absd{c}")
                        nc.scalar.activation(
                            out=absd[:], in_=diff[:],
                            func=mybir.ActivationFunctionType.Abs,
                            accum_out=acol,
                        )

            if m_tiles:
                evac = singles.tile([1, NMM], mybir.dt.float32)
                nc.scalar.copy(out=evac[:], in_=ps_acc[:])
                nc.sync.dma_start(out=pe_out[:], in_=evac[:])
            nc.sync.dma_start(out=out[:], in_=partials[:])
    return _split_multiwaits(nc)


def _prepare_in_maps(e_vectors, W, i):
    e = np.asarray(e_vectors, dtype=np.float32).reshape(B, K, D)
    idx = np.asarray(i).astype(np.int64)
    target = np.ascontiguousarray(W[:, idx].T)  # [B, D] f32, target[b] = W[:, i[b]]

    # [core, t, b_local, j, d] so device rows are tile-major with p = 4*b + j.
    e_bf = (
        e.reshape(NCORES, BPC, K // 4, 4, D)
        .transpose(0, 2, 1, 3, 4)
        .reshape(NCORES, ROWS, D)
        .astype(ml_dtypes.bfloat16)
    )
    t_bf = target.astype(ml_dtypes.bfloat16)

    in_maps = []
    for c in range(NCORES):
        t_rep = np.repeat(t_bf[c * BPC:(c + 1) * BPC], 4, axis=0)  # [128, D]
        in_maps.append({
            "e": np.ascontiguousarray(e_bf[c]),
            "trep": np.ascontiguousarray(t_rep),
        })
    return in_maps


def _run(e_vectors, W, i, **spmd_kwargs):
    """Returns (loss: np.float32, BassKernelResults)."""
    global _cached
    from concourse.bass_utils import run_bass_kernel_spmd

    if _cached is None:
        _cached = _build_nc()
    in_maps = _prepare_in_maps(e_vectors, W, i)
    res = run_bass_kernel_spmd(_cached, in_maps, core_ids=list(range(NCORES)), **spmd_kwargs)
    total = 0.0
    for r in res.results:
        total += np.asarray(r["partials"], dtype=np.float64).sum()
        if "pe_out" in r:
            total += np.asarray(r["pe_out"], dtype=np.float64).sum()
    loss = MATCH_WEIGHT * total / float(B * K * D)
    return np.float32(loss), res


def kernel(e_vectors, W, i):
    loss, _ = _run(e_vectors, W, i)
    return loss
